# revision 10
# baseline (speedup 1.0000x reference)
"""Trainium2 Bass kernel for nn_Canvas_DIP_by_distance (vq_codebook).

reference semantics:
  weight = sigmoid(weight_logits)                       (224, 224, 3)
  d[h,w,c] = sum_k (palette[c,k] - weight[h,w,k])^2     (224, 224, 64)
  idx = argmax_c softmax(d + 1) = argmax_c d
  colors[ch,h,w] = palette[idx[h,w], ch]                (3, 224, 224)
  out = nearest_upsample(colors, 2048, 2048)            (3, 2048, 2048)

argmax_c d == argmax_c v where v[c] = 0.5*sum_k p[c,k]^2 - sum_k p[c,k]*w[k]
(per-pixel |w|^2 term is constant in c). v is computed on the PE as a K=4
matmul with an augmented weight row of ones carrying the 0.5*|p_c|^2 bias.

Sharding: canvas rows split across 8 cores (28 rows -> 256 output rows each).
"""

import numpy as np
from contextlib import ExitStack

CANVAS_H, CANVAS_W, NUM_COLORS = 224, 224, 64
IMAGE_H = IMAGE_W = 2048
N_CORES = 8
HC = CANVAS_H // N_CORES          # 28 canvas rows per core
ORC = IMAGE_H // N_CORES          # 256 output rows per core
WH = CANVAS_W // 2                # 112, w-half (matmul K limit is 128)

# nearest-upsample row/col map: src = (dst * 224) // 2048 = (7*dst) // 64.
# per 7 consecutive src indices the replication widths are [10,9,9,9,9,9,9].
_OFF = [0, 10, 19, 28, 37, 46, 55]     # first dst index for src%7 == b (mod 64)
_WIDTH = [10, 9, 9, 9, 9, 9, 9]

_CACHE = {}


def _build_program(debug=False):
    """Build (once) the Bass program; returns (nc, input names)."""
    import concourse.bacc as bacc
    import concourse.tile as tile
    import concourse.mybir as mybir
    from concourse import bass

    f32 = mybir.dt.float32
    nc = bacc.Bacc("TRN2", target_bir_lowering=False)

    w_in = nc.dram_tensor("w_in", [HC, CANVAS_W, 3], f32, kind="ExternalInput")
    b4_in = nc.dram_tensor("b4_in", [WH, HC * 64], f32, kind="ExternalInput")
    prep_in = nc.dram_tensor("prep_in", [128, 3], f32, kind="ExternalInput")
    e_in = nc.dram_tensor("e_in", [2, WH, IMAGE_W], f32, kind="ExternalInput")
    id_in = nc.dram_tensor("id_in", [128, 128], f32, kind="ExternalInput")
    out = nc.dram_tensor("out", [3, ORC, IMAGE_W], f32, kind="ExternalOutput")
    dbg = {}
    if debug:
        dbg["s"] = nc.dram_tensor("dbg_s", [WH, 2, HC, NUM_COLORS], f32,
                                  kind="ExternalOutput")
        dbg["oh"] = nc.dram_tensor("dbg_oh", [WH, 2, HC, NUM_COLORS], f32,
                                   kind="ExternalOutput")
        dbg["cw"] = nc.dram_tensor("dbg_cw", [WH, 2, HC, 3], f32,
                                   kind="ExternalOutput")
        dbg["expd"] = nc.dram_tensor("dbg_expd", [84, IMAGE_W], f32,
                                     kind="ExternalOutput")
        dbg["w4g0"] = nc.dram_tensor("dbg_w4g0", [112, WH], f32,
                                     kind="ExternalOutput")

    with tile.TileContext(nc) as tc:
        with ExitStack() as ctx:
            sb = ctx.enter_context(tc.tile_pool(name="sb", bufs=1))
            ps = ctx.enter_context(tc.tile_pool(name="ps", bufs=2, space="PSUM"))

            # ---- constants into SBUF -------------------------------------
            b4 = sb.tile([WH, HC * 64], f32, tag="b4")
            nc.sync.dma_start(out=b4[:], in_=b4_in[:])
            prep = sb.tile([128, 3], f32, tag="prep")
            nc.sync.dma_start(out=prep[:], in_=prep_in[:])
            ident = sb.tile([128, 128], f32, tag="ident")
            nc.sync.dma_start(out=ident[:], in_=id_in[:])
            esb = sb.tile([WH, 2, IMAGE_W], f32, tag="esb")
            for hf in range(2):
                nc.sync.dma_start(out=esb[:, hf], in_=e_in[hf])

            # ---- load canvas slice in w-major layout ---------------------
            # wsrc[w', hf, h, k] = w_in[h, hf*112 + w', k]
            wsrc = w_in[:].rearrange("h (f w) k -> w f h k", f=2)
            wraw = sb.tile([WH, 2, HC, 3], f32, tag="wraw")
            for hf in range(2):
                nc.sync.dma_start(out=wraw[:, hf], in_=wsrc[:, hf])

            # ---- sigmoid + augmented ones row ----------------------------
            waug = sb.tile([WH, 2, HC, 4], f32, tag="waug")
            for hf in range(2):
                nc.scalar.activation(
                    out=waug[:, hf, :, 0:3], in_=wraw[:, hf],
                    func=mybir.ActivationFunctionType.Sigmoid,
                )
            nc.vector.memset(waug[:, :, :, 3:4], 1.0)

            # ---- PE transpose -> W4g[4h+k, w] per half -------------------
            w4g = []
            for hf in range(2):
                tp = ps.tile([112, WH], f32, tag="tr")
                nc.tensor.transpose(
                    out=tp[:], in_=waug[:, hf].rearrange("w h k -> w (h k)"),
                    identity=ident[0:WH, 0:112],
                )
                g = sb.tile([112, WH], f32, tag=f"w4g{hf}")
                nc.vector.tensor_copy(out=g[:], in_=tp[:])
                w4g.append(g)

            # ---- v = 0.5|p|^2 - p.w  via block-diagonal matmuls ----------
            # b4 is block diagonal over h: rows (28h x 4k), cols (28h x 64c),
            # so a K=112 matmul computes v for all rows; N-chunked by 512.
            s = sb.tile([WH, 2, HC, NUM_COLORS], f32, tag="s")
            for hf in range(2):
                for g in range(4):
                    nh = min(8, HC - 8 * g)
                    nn = 64 * nh
                    sp = ps.tile([WH, 512], f32, tag="sps")
                    nc.tensor.matmul(
                        out=sp[:, 0:nn],
                        lhsT=w4g[hf][:],
                        rhs=b4[:, 512 * g:512 * g + nn],
                        start=True, stop=True,
                    )
                    eng = nc.vector if (g % 2 == 0) else nc.scalar
                    dst = s[:, hf, 8 * g:8 * g + nh].rearrange("w h c -> w (h c)")
                    if eng is nc.vector:
                        eng.tensor_copy(out=dst, in_=sp[:, 0:nn])
                    else:
                        eng.copy(out=dst, in_=sp[:, 0:nn])

            vmax = sb.tile([WH, 2, HC], f32, tag="vmax")
            nc.vector.reduce_max(out=vmax[:], in_=s[:], axis=mybir.AxisListType.X)
            oh = sb.tile([WH, 2, HC, NUM_COLORS], f32, tag="oh")
            nc.vector.tensor_tensor(
                out=oh[:], in0=s[:],
                in1=vmax[:].unsqueeze(3).to_broadcast([WH, 2, HC, NUM_COLORS]),
                op=mybir.AluOpType.is_equal,
            )

            # ---- transpose onehot (pairs of h) then colors via palette ---
            # oht[:, hf, j, :]: partition (dh, c) = dh*64 + c; free w
            oht = sb.tile([128, 2, HC // 2, WH], f32, tag="oht")
            for hf in range(2):
                for j in range(HC // 2):
                    tp = ps.tile([128, WH], f32, tag="tr")
                    nc.tensor.transpose(
                        out=tp[:],
                        in_=oh[:, hf, 2 * j:2 * j + 2].rearrange("w h c -> w (h c)"),
                        identity=ident[0:WH, 0:112],
                    )
                    eng = nc.vector if (j % 2 == 0) else nc.scalar
                    if eng is nc.vector:
                        eng.tensor_copy(out=oht[:, hf, j], in_=tp[:])
                    else:
                        eng.copy(out=oht[:, hf, j], in_=tp[:])

            # colors cw[:, hf, pos, ch], pos = 4*b + a for h = 7*a + b
            cw = sb.tile([WH, 2, HC, 3], f32, tag="cw")
            for hf in range(2):
                for h in range(HC):
                    a, b = h // 7, h % 7
                    pos = 4 * b + a
                    cp = ps.tile([WH, 3], f32, tag="cps")
                    base = 64 * (h % 2)
                    nc.tensor.matmul(
                        out=cp[:],
                        lhsT=oht[base:base + 64, hf, h // 2, :],
                        rhs=prep[base:base + 64, :],
                        start=True, stop=True,
                    )
                    eng = nc.vector if (h % 2 == 0) else nc.scalar
                    if eng is nc.vector:
                        eng.tensor_copy(out=cw[:, hf, pos], in_=cp[:])
                    else:
                        eng.copy(out=cw[:, hf, pos], in_=cp[:])

            # ---- column expansion: exp[(b,a,ch), j] ----------------------
            expd = sb.tile([84, IMAGE_W], f32, tag="expd")
            NCH = 4
            for jc in range(NCH):
                ep = ps.tile([84, IMAGE_W // NCH], f32, tag="exps")
                sl = slice(jc * (IMAGE_W // NCH), (jc + 1) * (IMAGE_W // NCH))
                for hf in range(2):
                    nc.tensor.matmul(
                        out=ep[:],
                        lhsT=cw[:, hf].rearrange("w p c -> w (p c)"),
                        rhs=esb[:, hf, sl],
                        start=(hf == 0), stop=(hf == 1),
                    )
                eng = nc.vector if (jc % 2 == 0) else nc.scalar
                if eng is nc.vector:
                    eng.tensor_copy(out=expd[:, sl], in_=ep[:])
                else:
                    eng.copy(out=expd[:, sl], in_=ep[:])

            if debug:
                nc.sync.dma_start(out=dbg["s"][:], in_=s[:])
                nc.sync.dma_start(out=dbg["oh"][:], in_=oh[:])
                nc.sync.dma_start(out=dbg["cw"][:], in_=cw[:])
                nc.sync.dma_start(out=dbg["expd"][:], in_=expd[:])
                nc.sync.dma_start(out=dbg["w4g0"][:], in_=w4g[0][:])

            # ---- row-replicating stores ----------------------------------
            # expd partition (b, a, ch) -> out[ch, 64*a + _OFF[b] + r, :]
            outv = out[:].rearrange("c (a i) j -> a c i j", a=4)
            for b in range(7):
                for r in range(_WIDTH[b]):
                    nc.sync.dma_start(
                        out=outv[:, :, _OFF[b] + r, :],
                        in_=expd[12 * b:12 * b + 12, :])

    nc.compile()
    return nc, ["w_in", "b4_in", "prep_in", "e_in", "id_in"]


def _host_consts(palette: np.ndarray):
    pal = palette.astype(np.float32)
    # block-diagonal distance matrix: rows (8h x 4k), cols (8h x 64c);
    # block[k<3, c] = -pal[c,k], block[3, c] = 0.5*|pal_c|^2
    row = np.empty((4, NUM_COLORS), np.float32)
    row[0:3] = -pal.T
    row[3] = 0.5 * (pal.astype(np.float64) ** 2).sum(-1).astype(np.float32)
    b4 = np.zeros((WH, HC * 64), np.float32)
    for h in range(HC):
        b4[4 * h:4 * h + 4, 64 * h:64 * h + 64] = row  # (112, 1792)
    prep = np.tile(pal, (2, 1))                      # (128, 3)
    # column-expansion matrix, split into two K-halves
    wmap = (np.arange(IMAGE_W) * CANVAS_W) // IMAGE_W
    e_full = (wmap[None, :] == np.arange(CANVAS_W)[:, None]).astype(np.float32)
    e = np.stack([e_full[:WH], e_full[WH:]])         # (2, 112, 2048)
    ident = np.eye(128, dtype=np.float32)
    return b4, prep, e, ident


def kernel(weight_logits, palette, image_h, image_w):
    weight_logits = np.asarray(weight_logits, np.float32)
    palette = np.asarray(palette, np.float32)
    assert int(image_h) == IMAGE_H and int(image_w) == IMAGE_W
    assert weight_logits.shape == (CANVAS_H, CANVAS_W, 3)

    if "nc" not in _CACHE:
        _CACHE["nc"] = _build_program()
    nc, in_names = _CACHE["nc"]

    from concourse import bass_utils

    b4, prep, e, ident = _host_consts(palette)
    in_maps = []
    for core in range(N_CORES):
        sl = weight_logits[core * HC:(core + 1) * HC]
        in_maps.append({
            "w_in": np.ascontiguousarray(sl),
            "b4_in": b4, "prep_in": prep, "e_in": e, "id_in": ident,
        })
    res = bass_utils.run_bass_kernel_spmd(
        nc, in_maps, core_ids=list(range(N_CORES)))
    outs = [res.results[c]["out"] for c in range(N_CORES)]
    return np.concatenate(outs, axis=1)


# revision 14
# speedup vs baseline: 1.1855x; 1.1855x over previous
"""Trainium2 Bass kernel for nn_Canvas_DIP_by_distance (vq_codebook).

reference semantics:
  weight = sigmoid(weight_logits)                       (224, 224, 3)
  d[h,w,c] = sum_k (palette[c,k] - weight[h,w,k])^2     (224, 224, 64)
  idx = argmax_c softmax(d + 1) = argmax_c d
  colors[ch,h,w] = palette[idx[h,w], ch]                (3, 224, 224)
  out = nearest_upsample(colors, 2048, 2048)            (3, 2048, 2048)

argmax_c d == argmax_c v where v[c] = 0.5*sum_k p[c,k]^2 - sum_k p[c,k]*w[k]
(the per-pixel |w|^2 term is constant in c).

Layout strategy (per core, 28 canvas rows -> 256 output rows):
  - canvas loaded w-major: [112 w-partitions, half, 28 h, ch]
  - v via one PE transpose + block-diagonal K=112 matmuls
  - one-hot via reduce_max + is_equal on the vector engine
  - colors^T via palette-stationary matmuls, small PE transposes back to
    w-partitions
  - column expansion (224 -> 2048) via matmul with 0/1 matrix E built
    on-chip by gpsimd affine_select
  - row replication (28 -> 256) via matmul with a 0/1 replication matrix, so
    the full 6 MB core output is materialized in SBUF and stored with six
    1 MB DMAs (store count, not bytes, dominated the v1 profile)
"""

import numpy as np
from contextlib import ExitStack

CANVAS_H, CANVAS_W, NUM_COLORS = 224, 224, 64
IMAGE_H = IMAGE_W = 2048
N_CORES = 8
HC = CANVAS_H // N_CORES          # 28 canvas rows per core
ORC = IMAGE_H // N_CORES          # 256 output rows per core
WH = CANVAS_W // 2                # 112, w-half (matmul K limit is 128)

_CACHE = {}


def _build_program(debug=False):
    import concourse.bacc as bacc
    import concourse.tile as tile
    import concourse.mybir as mybir
    from concourse import bass

    f32 = mybir.dt.float32
    ALU = mybir.AluOpType
    nc = bacc.Bacc("TRN2", target_bir_lowering=False)

    w_in = nc.dram_tensor("w_in", [HC, CANVAS_W, 3], f32, kind="ExternalInput")
    b4_in = nc.dram_tensor("b4_in", [WH, HC * NUM_COLORS], f32,
                           kind="ExternalInput")
    prep_in = nc.dram_tensor("prep_in", [128, 3], f32, kind="ExternalInput")
    rt3_in = nc.dram_tensor("rt3_in", [96, ORC], f32, kind="ExternalInput")
    id_in = nc.dram_tensor("id_in", [128, 128], f32, kind="ExternalInput")
    e_in = nc.dram_tensor("e_in", [WH, 2, IMAGE_W], f32, kind="ExternalInput")
    out = nc.dram_tensor("out", [3, ORC, IMAGE_W], f32, kind="ExternalOutput")
    dbg = {}
    if debug:
        dbg["expd"] = nc.dram_tensor("dbg_expd", [96, IMAGE_W], f32,
                                     kind="ExternalOutput")
        dbg["cw"] = nc.dram_tensor("dbg_cw", [WH, 2, 3, 32], f32,
                                   kind="ExternalOutput")

    with tile.TileContext(nc) as tc:
        with ExitStack() as ctx:
            sb = ctx.enter_context(tc.tile_pool(name="sb", bufs=1))
            ps = ctx.enter_context(tc.tile_pool(name="ps", bufs=1, space="PSUM"))

            # ---- small constants ----------------------------------------
            prep = sb.tile([128, 3], f32, tag="prep")
            nc.sync.dma_start(out=prep[:], in_=prep_in[:])

            rt3 = sb.tile([96, ORC], f32, tag="rt3")
            nc.sync.dma_start(out=rt3[:], in_=rt3_in[:])

            ident = sb.tile([128, 128], f32, tag="ident")
            nc.sync.dma_start(out=ident[:], in_=id_in[:])

            # block-diagonal distance matrix b4[4h+k, 64h+c]
            b4 = sb.tile([WH, HC * NUM_COLORS], f32, tag="b4")
            nc.sync.dma_start(out=b4[:], in_=b4_in[:])

            # column-expansion matrix (0/1), loaded late: only mmA needs it
            esb = sb.tile([WH, 2, IMAGE_W], f32, tag="esb")
            for hf in range(2):
                nc.sync.dma_start(out=esb[:, hf], in_=e_in[:, hf])

            # ---- canvas slice, w-major ----------------------------------
            wsrc = w_in[:].rearrange("h (f w) k -> w f h k", f=2)
            wraw = sb.tile([WH, 2, HC, 3], f32, tag="wraw")
            for hf in range(2):
                nc.sync.dma_start(out=wraw[:, hf], in_=wsrc[:, hf])

            waug = sb.tile([WH, 2, HC, 4], f32, tag="waug")
            for hf in range(2):
                nc.scalar.activation(
                    out=waug[:, hf, :, 0:3], in_=wraw[:, hf],
                    func=mybir.ActivationFunctionType.Sigmoid)
            nc.vector.memset(waug[:, :, :, 3:4], 1.0)

            # ---- W4g[4h+k, w] per half via PE transpose -----------------
            w4g = []
            for hf in range(2):
                tp = ps.tile([112, WH], f32, tag="psA", bufs=2)
                nc.tensor.transpose(
                    out=tp[:], in_=waug[:, hf].rearrange("w h k -> w (h k)"),
                    identity=ident[0:WH, 0:112])
                g = sb.tile([112, WH], f32, tag=f"w4g{hf}")
                nc.vector.tensor_copy(out=g[:], in_=tp[:])
                w4g.append(g)

            # ---- v via block-diagonal matmuls ---------------------------
            s = sb.tile([WH, 2, HC, NUM_COLORS], f32, tag="s")
            for hf in range(2):
                for g in range(4):
                    nh = min(8, HC - 8 * g)
                    nn = 64 * nh
                    sp = ps.tile([WH, 512], f32, tag="psB", bufs=4)
                    nc.tensor.matmul(
                        out=sp[:, 0:nn], lhsT=w4g[hf][:],
                        rhs=b4[:, 512 * g:512 * g + nn],
                        start=True, stop=True)
                    eng = nc.vector if (g % 2 == 0) else nc.scalar
                    dst = s[:, hf, 8 * g:8 * g + nh].rearrange("w h c -> w (h c)")
                    if eng is nc.vector:
                        eng.tensor_copy(out=dst, in_=sp[:, 0:nn])
                    else:
                        eng.copy(out=dst, in_=sp[:, 0:nn])

            # ---- argmax one-hot -----------------------------------------
            vmax = sb.tile([WH, 2, HC], f32, tag="vmax")
            nc.vector.reduce_max(out=vmax[:], in_=s[:], axis=mybir.AxisListType.X)
            oh = sb.tile([WH, 2, HC, NUM_COLORS], f32, tag="oh")
            nc.vector.tensor_tensor(
                out=oh[:], in0=s[:],
                in1=vmax[:].unsqueeze(3).to_broadcast([WH, 2, HC, NUM_COLORS]),
                op=ALU.is_equal)

            # ---- transpose one-hot: oht[64*dh + c, hf, j, w], h = 2j+dh -
            oht = sb.tile([128, 2, HC // 2, WH], f32, tag="oht")
            for hf in range(2):
                for j in range(HC // 2):
                    tp = ps.tile([128, WH], f32, tag="psA", bufs=2)
                    nc.tensor.transpose(
                        out=tp[:],
                        in_=oh[:, hf, 2 * j:2 * j + 2].rearrange("w h c -> w (h c)"),
                        identity=ident[0:WH, 0:112])
                    eng = nc.vector if (j % 2 == 0) else nc.scalar
                    if eng is nc.vector:
                        eng.tensor_copy(out=oht[:, hf, j], in_=tp[:])
                    else:
                        eng.copy(out=oht[:, hf, j], in_=tp[:])

            # ---- colors^T: palette stationary, one-hot moving ----------
            # ctsb[ch, hf, h, w] = palette[idx, ch]; batches of 4 same-parity h
            ctsb = sb.tile([3, 2, HC, WH], f32, tag="ctsb")
            for hf in range(2):
                for dh in range(2):
                    for jc in range(4):
                        j0 = 4 * jc
                        nj = min(4, HC // 2 - j0)
                        cp = ps.tile([3, 448], f32, tag="psB", bufs=4)
                        nc.tensor.matmul(
                            out=cp[:, 0:nj * WH],
                            lhsT=prep[64 * dh:64 * dh + 64, :],
                            rhs=oht[64 * dh:64 * dh + 64, hf, j0:j0 + nj]
                            .rearrange("c j w -> c (j w)"),
                            start=True, stop=True)
                        eng = nc.vector if (jc % 2 == 0) else nc.scalar
                        dst = ctsb[:, hf, 2 * j0 + dh:2 * (j0 + nj - 1) + dh + 1:2]
                        src3 = cp[:, 0:nj * WH].rearrange("c (j w) -> c j w", w=WH)
                        if eng is nc.vector:
                            eng.tensor_copy(out=dst, in_=src3)
                        else:
                            eng.copy(out=dst, in_=src3)

            # ---- colors back to w-partitions: cw[w, hf, ch, pos] --------
            # pos = 4*b + a for h = 7*a + b (row-block-major ordering)
            cw = sb.tile([WH, 2, 3, 32], f32, tag="cw")
            nc.vector.memset(cw[:], 0.0)
            for hf in range(2):
                for h in range(HC):
                    a, b = h // 7, h % 7
                    pos = 4 * b + a
                    tp = ps.tile([WH, 3], f32, tag="psA", bufs=2)
                    nc.tensor.transpose(
                        out=tp[:], in_=ctsb[:, hf, h, :],
                        identity=ident[0:3, 0:3])
                    eng = nc.vector if (h % 2 == 0) else nc.scalar
                    dst = cw[:, hf, :, pos].unsqueeze(2)
                    if eng is nc.vector:
                        eng.tensor_copy(out=dst, in_=tp[:].unsqueeze(2))
                    else:
                        eng.copy(out=dst, in_=tp[:].unsqueeze(2))

            # ---- column expansion: expd[(ch, pos), j] ------------------
            expd = sb.tile([96, IMAGE_W], f32, tag="expd")
            NCH = 4
            for jc in range(NCH):
                ep = ps.tile([96, IMAGE_W // NCH], f32, tag="psB", bufs=4)
                sl = slice(jc * (IMAGE_W // NCH), (jc + 1) * (IMAGE_W // NCH))
                for hf in range(2):
                    nc.tensor.matmul(
                        out=ep[:],
                        lhsT=cw[:, hf].rearrange("w c p -> w (c p)"),
                        rhs=esb[:, hf, sl],
                        start=(hf == 0), stop=(hf == 1))
                eng = nc.vector if (jc % 2 == 0) else nc.scalar
                if eng is nc.vector:
                    eng.tensor_copy(out=expd[:, sl], in_=ep[:])
                else:
                    eng.copy(out=expd[:, sl], in_=ep[:])

            if debug:
                nc.sync.dma_start(out=dbg["expd"][:], in_=expd[:])
                nc.sync.dma_start(out=dbg["cw"][:], in_=cw[:])

            # ---- row replication to full 256 rows, then 6 1MB stores ---
            NRH = 512
            for ch in range(3):
                for hf2 in range(2):
                    of = sb.tile([128, IMAGE_W], f32, tag=f"of{ch}{hf2}")
                    for jc in range(IMAGE_W // NRH):
                        rp = ps.tile([128, NRH], f32, tag="psB", bufs=4)
                        sl = slice(jc * NRH, (jc + 1) * NRH)
                        nc.tensor.matmul(
                            out=rp[:],
                            lhsT=rt3[32 * ch:32 * ch + 28,
                                     128 * hf2:128 * hf2 + 128],
                            rhs=expd[32 * ch:32 * ch + 28, sl],
                            start=True, stop=True)
                        eng = nc.vector if (jc % 2 == 0) else nc.scalar
                        if eng is nc.vector:
                            eng.tensor_copy(out=of[:, sl], in_=rp[:])
                        else:
                            eng.copy(out=of[:, sl], in_=rp[:])
                    dma_eng = nc.sync if ((ch + hf2) % 2 == 0) else nc.scalar
                    dma_eng.dma_start(
                        out=out[ch, 128 * hf2:128 * hf2 + 128, :], in_=of[:])

    nc.compile()
    return nc, ["w_in", "b4_in", "prep_in", "rt3_in", "id_in", "e_in"]


def _host_consts(palette: np.ndarray):
    pal = palette.astype(np.float32)
    # block-diagonal distance matrix: rows (28h x 4k), cols (28h x 64c)
    b4row = np.empty((4, NUM_COLORS), np.float32)
    b4row[0:3] = -pal.T
    b4row[3] = 0.5 * (pal.astype(np.float64) ** 2).sum(-1).astype(np.float32)
    b4 = np.zeros((WH, HC * NUM_COLORS), np.float32)
    for h in range(HC):
        b4[4 * h:4 * h + 4, 64 * h:64 * h + 64] = b4row
    prep = np.tile(pal, (2, 1))                      # (128, 3)
    # row-replication matrix, tripled so lhsT/rhs share base partitions
    rowmap = (np.arange(ORC) * CANVAS_H) // IMAGE_H  # canvas row per out row
    posmap = 4 * (rowmap % 7) + rowmap // 7
    rt = (posmap[None, :] == np.arange(32)[:, None]).astype(np.float32)
    rt3 = np.concatenate([rt, rt, rt], axis=0)       # (96, 256)
    # column-expansion matrix, w split into two K-halves on dim 0
    wmap = (np.arange(IMAGE_W) * CANVAS_W) // IMAGE_W
    e_full = (wmap[None, :] == np.arange(CANVAS_W)[:, None]).astype(np.float32)
    e = np.ascontiguousarray(
        np.stack([e_full[:WH], e_full[WH:]], axis=1))  # (112, 2, 2048)
    ident = np.eye(128, dtype=np.float32)
    return b4, prep, rt3, e, ident


def kernel(weight_logits, palette, image_h, image_w):
    weight_logits = np.asarray(weight_logits, np.float32)
    palette = np.asarray(palette, np.float32)
    assert int(image_h) == IMAGE_H and int(image_w) == IMAGE_W
    assert weight_logits.shape == (CANVAS_H, CANVAS_W, 3)

    if "nc" not in _CACHE:
        _CACHE["nc"] = _build_program()
    nc, _ = _CACHE["nc"]

    from concourse import bass_utils

    b4, prep, rt3, e, ident = _host_consts(palette)
    in_maps = []
    for core in range(N_CORES):
        sl = weight_logits[core * HC:(core + 1) * HC]
        in_maps.append({
            "w_in": np.ascontiguousarray(sl),
            "b4_in": b4, "prep_in": prep, "rt3_in": rt3,
            "id_in": ident, "e_in": e,
        })
    res = bass_utils.run_bass_kernel_spmd(
        nc, in_maps, core_ids=list(range(N_CORES)))
    outs = [res.results[c]["out"] for c in range(N_CORES)]
    return np.concatenate(outs, axis=1)


# revision 16
# speedup vs baseline: 1.7046x; 1.4378x over previous
"""Trainium2 Bass kernel for nn_Canvas_DIP_by_distance (vq_codebook).

reference semantics:
  weight = sigmoid(weight_logits)                       (224, 224, 3)
  d[h,w,c] = sum_k (palette[c,k] - weight[h,w,k])^2     (224, 224, 64)
  idx = argmax_c softmax(d + 1) = argmax_c d
  colors[ch,h,w] = palette[idx[h,w], ch]                (3, 224, 224)
  out = nearest_upsample(colors, 2048, 2048)            (3, 2048, 2048)

argmax_c d == argmax_c v where v[c] = 0.5*sum_k p[c,k]^2 - sum_k p[c,k]*w[k]
(the per-pixel |w|^2 term is constant in c). The argmax matmul stays fp32.

All palette-value-carrying matmuls (palette apply, column expansion, row
replication) run as fp16 hi/lo two-splits: x = fp16(x) + fp16(x - fp16(x))
reconstructs fp32 to <= 1 ulp, the 0/1 selection matrices are exact in fp16,
and fp16 streams 4x faster through the PE than fp32 (4 cy/row -> 1 cy/row).

Per core (28 canvas rows -> 256 output rows):
  - canvas loaded w-major [112 w-partitions, half, 28 h, ch]
  - v via one fp32 PE transpose + block-diagonal K=112 fp32 matmuls
  - one-hot (fp16) via reduce_max + is_equal
  - colors^T via palette-stationary fp16 split matmuls; tiny PE transposes
    back to w-partitions (batched into shared PSUM tiles by row-block)
  - column expansion via fp16 split matmuls against 0/1 E
  - row replication via fp16 split matmuls against 0/1 RT, materializing the
    full 2 MB per-channel output in SBUF, stored with one DMA per channel
"""

import numpy as np
from contextlib import ExitStack

CANVAS_H, CANVAS_W, NUM_COLORS = 224, 224, 64
IMAGE_H = IMAGE_W = 2048
N_CORES = 8
HC = CANVAS_H // N_CORES          # 28 canvas rows per core
ORC = IMAGE_H // N_CORES          # 256 output rows per core
WH = CANVAS_W // 2                # 112, w-half (matmul K limit is 128)

_CACHE = {}


def _build_program(debug=False):
    import concourse.bacc as bacc
    import concourse.tile as tile
    import concourse.mybir as mybir
    from concourse import bass

    f32 = mybir.dt.float32
    f16 = mybir.dt.float16
    ALU = mybir.AluOpType
    nc = bacc.Bacc("TRN2", target_bir_lowering=False)

    w_in = nc.dram_tensor("w_in", [HC, CANVAS_W, 3], f32, kind="ExternalInput")
    b4_in = nc.dram_tensor("b4_in", [WH, HC * 64], f32, kind="ExternalInput")
    prep_in = nc.dram_tensor("prep_in", [128, 2, 3], f16, kind="ExternalInput")
    rt3_in = nc.dram_tensor("rt3_in", [96, ORC], f16, kind="ExternalInput")
    id_in = nc.dram_tensor("id_in", [128, 128], f32, kind="ExternalInput")
    e_in = nc.dram_tensor("e_in", [WH, 2, IMAGE_W], f16, kind="ExternalInput")
    out = nc.dram_tensor("out", [3, ORC, IMAGE_W], f32, kind="ExternalOutput")
    dbg = {}
    if debug:
        dbg["expd"] = nc.dram_tensor("dbg_expd", [96, IMAGE_W], f32,
                                     kind="ExternalOutput")

    with tile.TileContext(nc) as tc:
        with ExitStack() as ctx:
            sb = ctx.enter_context(tc.tile_pool(name="sb", bufs=1))
            ps = ctx.enter_context(tc.tile_pool(name="ps", bufs=1, space="PSUM"))

            # ---- constants ----------------------------------------------
            prep = sb.tile([128, 2, 3], f16, tag="prep")
            nc.sync.dma_start(out=prep[:], in_=prep_in[:])
            ident = sb.tile([128, 128], f32, tag="ident")
            nc.sync.dma_start(out=ident[:], in_=id_in[:])
            b4 = sb.tile([WH, HC * 64], f32, tag="b4")
            nc.sync.dma_start(out=b4[:], in_=b4_in[:])
            rt3 = sb.tile([96, ORC], f16, tag="rt3")
            nc.sync.dma_start(out=rt3[:], in_=rt3_in[:])
            esb = sb.tile([WH, 2, IMAGE_W], f16, tag="esb")
            for hf in range(2):
                nc.sync.dma_start(out=esb[:, hf], in_=e_in[:, hf])
            ident16 = sb.tile([112, 112], f16, tag="ident16")
            nc.vector.tensor_copy(out=ident16[:], in_=ident[0:112, 0:112])

            # ---- canvas slice, w-major ----------------------------------
            wsrc = w_in[:].rearrange("h (f w) k -> w f h k", f=2)
            wraw = sb.tile([WH, 2, HC, 3], f32, tag="wraw")
            for hf in range(2):
                nc.sync.dma_start(out=wraw[:, hf], in_=wsrc[:, hf])

            waug = sb.tile([WH, 2, HC, 4], f32, tag="waug")
            for hf in range(2):
                nc.scalar.activation(
                    out=waug[:, hf, :, 0:3], in_=wraw[:, hf],
                    func=mybir.ActivationFunctionType.Sigmoid)
            nc.vector.memset(waug[:, :, :, 3:4], 1.0)

            # ---- W4g[4h+k, w] per half via fp32 PE transpose ------------
            w4g = []
            for hf in range(2):
                tp = ps.tile([112, WH], f32, tag="psA", bufs=2)
                nc.tensor.transpose(
                    out=tp[:], in_=waug[:, hf].rearrange("w h k -> w (h k)"),
                    identity=ident[0:WH, 0:112])
                g = sb.tile([112, WH], f32, tag=f"w4g{hf}")
                nc.vector.tensor_copy(out=g[:], in_=tp[:])
                w4g.append(g)

            # ---- v via block-diagonal fp32 matmuls (argmax precision) ---
            s = sb.tile([WH, 2, HC, NUM_COLORS], f32, tag="s")
            for hf in range(2):
                for g in range(4):
                    nh = min(8, HC - 8 * g)
                    nn = 64 * nh
                    sp = ps.tile([WH, 512], f32, tag="psS", bufs=2)
                    nc.tensor.matmul(
                        out=sp[:, 0:nn], lhsT=w4g[hf][:],
                        rhs=b4[:, 512 * g:512 * g + nn],
                        start=True, stop=True)
                    eng = nc.vector if (g % 2 == 0) else nc.scalar
                    dst = s[:, hf, 8 * g:8 * g + nh].rearrange("w h c -> w (h c)")
                    if eng is nc.vector:
                        eng.tensor_copy(out=dst, in_=sp[:, 0:nn])
                    else:
                        eng.copy(out=dst, in_=sp[:, 0:nn])

            # ---- argmax one-hot (fp16: 0/1 exact) -----------------------
            vmax = sb.tile([WH, 2, HC], f32, tag="vmax")
            nc.vector.reduce_max(out=vmax[:], in_=s[:], axis=mybir.AxisListType.X)
            oh = sb.tile([WH, 2, HC, NUM_COLORS], f16, tag="oh")
            nc.vector.tensor_tensor(
                out=oh[:], in0=s[:],
                in1=vmax[:].unsqueeze(3).to_broadcast([WH, 2, HC, NUM_COLORS]),
                op=ALU.is_equal)

            # ---- transpose one-hot (fp16): oht[64*dh + c, hf, j, w] -----
            oht = sb.tile([128, 2, HC // 2, WH], f16, tag="oht")
            for hf in range(2):
                for jc in range(4):
                    j0, nj = 4 * jc, min(4, HC // 2 - 4 * jc)
                    tp = ps.tile([128, 4 * WH], f16, tag="psA", bufs=2)
                    for j in range(j0, j0 + nj):
                        nc.tensor.transpose(
                            out=tp[:, (j - j0) * WH:(j - j0 + 1) * WH],
                            in_=oh[:, hf, 2 * j:2 * j + 2]
                            .rearrange("w h c -> w (h c)"),
                            identity=ident16[:, 0:112])
                    eng = nc.vector if (jc % 2 == 0) else nc.scalar
                    dst = (oht[:, hf, j0:j0 + nj]
                           .rearrange("c j w -> c (j w)"))
                    if eng is nc.vector:
                        eng.tensor_copy(out=dst, in_=tp[:, 0:nj * WH])
                    else:
                        eng.copy(out=dst, in_=tp[:, 0:nj * WH])

            # ---- colors^T via palette-stationary fp16 split matmuls -----
            # ctsb[ch, hf, h, w]; h = 2j + dh
            ctsb = sb.tile([3, 2, HC, WH], f32, tag="ctsb")
            for hf in range(2):
                for dh in range(2):
                    for jc in range(4):
                        j0, nj = 4 * jc, min(4, HC // 2 - 4 * jc)
                        cp = ps.tile([3, 448], f32, tag="psB", bufs=4)
                        rhs = (oht[64 * dh:64 * dh + 64, hf, j0:j0 + nj]
                               .rearrange("c j w -> c (j w)"))
                        for part in range(2):
                            nc.tensor.matmul(
                                out=cp[:, 0:nj * WH],
                                lhsT=prep[64 * dh:64 * dh + 64, part, :],
                                rhs=rhs,
                                start=(part == 0), stop=(part == 1))
                        eng = nc.vector if (jc % 2 == 0) else nc.scalar
                        dst = ctsb[:, hf, 2 * j0 + dh:2 * (j0 + nj - 1) + dh + 1:2]
                        src3 = cp[:, 0:nj * WH].rearrange("c (j w) -> c j w", w=WH)
                        if eng is nc.vector:
                            eng.tensor_copy(out=dst, in_=src3)
                        else:
                            eng.copy(out=dst, in_=src3)

            # ---- colors to w-partitions: cw[w, hf, ch, pos] -------------
            # pos = 4*b + a for h = 7*a + b; batch the 4 same-b transposes
            # into one PSUM tile so each copy covers 4 consecutive pos
            cw = sb.tile([WH, 2, 3, 32], f32, tag="cw")
            nc.vector.memset(cw[:], 0.0)
            for hf in range(2):
                for b in range(7):
                    tp = ps.tile([WH, 4, 3], f32, tag="psA", bufs=2)
                    for a in range(4):
                        h = 7 * a + b
                        nc.tensor.transpose(
                            out=tp[:, a, :], in_=ctsb[:, hf, h, :],
                            identity=ident[0:3, 0:3])
                    eng = nc.vector if (b % 2 == 0) else nc.scalar
                    dst = cw[:, hf, :, 4 * b:4 * b + 4]
                    src = tp[:].rearrange("w a c -> w c a")
                    if eng is nc.vector:
                        eng.tensor_copy(out=dst, in_=src)
                    else:
                        eng.copy(out=dst, in_=src)

            # fp16 hi/lo split of cw
            cw16 = sb.tile([WH, 2, 2, 3, 32], f16, tag="cw16")
            cwt = sb.tile([WH, 2, 3, 32], f32, tag="cwt")
            nc.vector.tensor_copy(out=cw16[:, :, 0], in_=cw[:])       # hi
            nc.scalar.copy(out=cwt[:], in_=cw16[:, :, 0])             # hi as f32
            nc.vector.tensor_sub(out=cwt[:], in0=cw[:], in1=cwt[:])   # lo f32
            nc.vector.tensor_copy(out=cw16[:, :, 1], in_=cwt[:])      # lo

            # ---- column expansion: expd[(ch, pos), j] ------------------
            expd = sb.tile([96, IMAGE_W], f32, tag="expd")
            for jc in range(4):
                ep = ps.tile([96, 512], f32, tag="psB", bufs=4)
                sl = slice(jc * 512, (jc + 1) * 512)
                for hf in range(2):
                    for part in range(2):
                        nc.tensor.matmul(
                            out=ep[:],
                            lhsT=cw16[:, hf, part].rearrange("w c p -> w (c p)"),
                            rhs=esb[:, hf, sl],
                            start=(hf == 0 and part == 0),
                            stop=(hf == 1 and part == 1))
                eng = nc.vector if (jc % 2 == 0) else nc.scalar
                if eng is nc.vector:
                    eng.tensor_copy(out=expd[:, sl], in_=ep[:])
                else:
                    eng.copy(out=expd[:, sl], in_=ep[:])

            if debug:
                nc.sync.dma_start(out=dbg["expd"][:], in_=expd[:])

            # fp16 hi/lo split of expd
            expd16 = sb.tile([96, 2, IMAGE_W], f16, tag="expd16")
            expt = sb.tile([96, IMAGE_W], f32, tag="expt")
            nc.vector.tensor_copy(out=expd16[:, 0], in_=expd[:])
            nc.scalar.copy(out=expt[:], in_=expd16[:, 0])
            nc.vector.tensor_sub(out=expt[:], in0=expd[:], in1=expt[:])
            nc.vector.tensor_copy(out=expd16[:, 1], in_=expt[:])

            # ---- row replication + one 2MB store per channel -----------
            # of[p, hf2, j] = out[ch, 128*hf2 + p, j]
            for ch in range(3):
                of = sb.tile([128, 2, IMAGE_W], f32, tag=f"of{ch}")
                for hf2 in range(2):
                    for jcc in range(4):
                        rp = ps.tile([128, 512], f32, tag="psB", bufs=4)
                        sl = slice(jcc * 512, (jcc + 1) * 512)
                        for part in range(2):
                            nc.tensor.matmul(
                                out=rp[:],
                                lhsT=rt3[32 * ch:32 * ch + 28,
                                         128 * hf2:128 * hf2 + 128],
                                rhs=expd16[32 * ch:32 * ch + 28, part, sl],
                                start=(part == 0), stop=(part == 1))
                        eng = nc.vector if (jcc % 2 == 0) else nc.scalar
                        if eng is nc.vector:
                            eng.tensor_copy(out=of[:, hf2, sl], in_=rp[:])
                        else:
                            eng.copy(out=of[:, hf2, sl], in_=rp[:])
                dma_eng = nc.sync if (ch % 2 == 0) else nc.scalar
                dma_eng.dma_start(
                    out=out[ch].rearrange("(f p) j -> p f j", f=2), in_=of[:])

    nc.compile()
    return nc, ["w_in", "b4_in", "prep_in", "rt3_in", "id_in", "e_in"]


def _host_consts(palette: np.ndarray):
    pal = palette.astype(np.float32)
    # block-diagonal distance matrix: rows (28h x 4k), cols (28h x 64c)
    b4row = np.empty((4, NUM_COLORS), np.float32)
    b4row[0:3] = -pal.T
    b4row[3] = 0.5 * (pal.astype(np.float64) ** 2).sum(-1).astype(np.float32)
    b4 = np.zeros((WH, HC * NUM_COLORS), np.float32)
    for h in range(HC):
        b4[4 * h:4 * h + 4, 64 * h:64 * h + 64] = b4row
    # palette hi/lo fp16 split, doubled along partitions
    hi = pal.astype(np.float16)
    lo = (pal - hi.astype(np.float32)).astype(np.float16)
    prep = np.stack([hi, lo], axis=1)                # (64, 2, 3)
    prep = np.concatenate([prep, prep], axis=0)      # (128, 2, 3)
    # row-replication matrix (0/1, fp16-exact), tripled for base partitions
    rowmap = (np.arange(ORC) * CANVAS_H) // IMAGE_H
    posmap = 4 * (rowmap % 7) + rowmap // 7
    rt = (posmap[None, :] == np.arange(32)[:, None]).astype(np.float16)
    rt3 = np.concatenate([rt, rt, rt], axis=0)       # (96, 256)
    # column-expansion matrix (0/1, fp16-exact), w split into two K-halves
    wmap = (np.arange(IMAGE_W) * CANVAS_W) // IMAGE_W
    e_full = (wmap[None, :] == np.arange(CANVAS_W)[:, None]).astype(np.float16)
    e = np.ascontiguousarray(
        np.stack([e_full[:WH], e_full[WH:]], axis=1))  # (112, 2, 2048)
    ident = np.eye(128, dtype=np.float32)
    return b4, prep, rt3, e, ident


def kernel(weight_logits, palette, image_h, image_w):
    weight_logits = np.asarray(weight_logits, np.float32)
    palette = np.asarray(palette, np.float32)
    assert int(image_h) == IMAGE_H and int(image_w) == IMAGE_W
    assert weight_logits.shape == (CANVAS_H, CANVAS_W, 3)

    if "nc" not in _CACHE:
        _CACHE["nc"] = _build_program()
    nc, _ = _CACHE["nc"]

    from concourse import bass_utils

    b4, prep, rt3, e, ident = _host_consts(palette)
    in_maps = []
    for core in range(N_CORES):
        sl = weight_logits[core * HC:(core + 1) * HC]
        in_maps.append({
            "w_in": np.ascontiguousarray(sl),
            "b4_in": b4, "prep_in": prep, "rt3_in": rt3,
            "id_in": ident, "e_in": e,
        })
    res = bass_utils.run_bass_kernel_spmd(
        nc, in_maps, core_ids=list(range(N_CORES)))
    outs = [res.results[c]["out"] for c in range(N_CORES)]
    return np.concatenate(outs, axis=1)


# revision 18
# speedup vs baseline: 1.7589x; 1.0319x over previous
"""Trainium2 Bass kernel for nn_Canvas_DIP_by_distance (vq_codebook).

reference semantics:
  weight = sigmoid(weight_logits)                       (224, 224, 3)
  d[h,w,c] = sum_k (palette[c,k] - weight[h,w,k])^2     (224, 224, 64)
  idx = argmax_c softmax(d + 1) = argmax_c d
  colors[ch,h,w] = palette[idx[h,w], ch]                (3, 224, 224)
  out = nearest_upsample(colors, 2048, 2048)            (3, 2048, 2048)

argmax_c d == argmax_c v where v[c] = 0.5*sum_k p[c,k]^2 - sum_k p[c,k]*w[k]
(the per-pixel |w|^2 term is constant in c). The argmax matmul stays fp32.

All palette-value-carrying matmuls (palette apply, column expansion, row
replication) run as fp16 hi/lo two-splits: x = fp16(x) + fp16(x - fp16(x))
reconstructs fp32 to <= 1 ulp, the 0/1 selection matrices are exact in fp16,
and fp16 streams 4x faster through the PE than fp32 (4 cy/row -> 1 cy/row).

Per core (28 canvas rows -> 256 output rows):
  - canvas loaded w-major [112 w-partitions, half, 28 h, ch]
  - v via one fp32 PE transpose + block-diagonal K=112 fp32 matmuls
  - one-hot (fp16) via reduce_max + is_equal
  - colors^T via palette-stationary fp16 split matmuls; tiny PE transposes
    back to w-partitions (batched into shared PSUM tiles by row-block)
  - column expansion via fp16 split matmuls against 0/1 E
  - row replication via fp16 split matmuls against 0/1 RT, materializing the
    full 2 MB per-channel output in SBUF, stored with one DMA per channel
"""

import numpy as np
from contextlib import ExitStack

CANVAS_H, CANVAS_W, NUM_COLORS = 224, 224, 64
IMAGE_H = IMAGE_W = 2048
N_CORES = 8
HC = CANVAS_H // N_CORES          # 28 canvas rows per core
ORC = IMAGE_H // N_CORES          # 256 output rows per core
WH = CANVAS_W // 2                # 112, w-half (matmul K limit is 128)

_CACHE = {}


def _build_program(debug=False):
    import concourse.bacc as bacc
    import concourse.tile as tile
    import concourse.mybir as mybir
    from concourse import bass

    f32 = mybir.dt.float32
    f16 = mybir.dt.float16
    ALU = mybir.AluOpType
    nc = bacc.Bacc("TRN2", target_bir_lowering=False)

    w_in = nc.dram_tensor("w_in", [HC, CANVAS_W, 3], f32, kind="ExternalInput")
    b4_in = nc.dram_tensor("b4_in", [WH, HC * 64], f32, kind="ExternalInput")
    prep_in = nc.dram_tensor("prep_in", [128, 2, 3], f16, kind="ExternalInput")
    rt3_in = nc.dram_tensor("rt3_in", [96, ORC], f16, kind="ExternalInput")
    id_in = nc.dram_tensor("id_in", [128, 128], f32, kind="ExternalInput")
    e_in = nc.dram_tensor("e_in", [WH, 2, IMAGE_W], f16, kind="ExternalInput")
    out = nc.dram_tensor("out", [3, ORC, IMAGE_W], f32, kind="ExternalOutput")
    dbg = {}
    if debug:
        dbg["expd"] = nc.dram_tensor("dbg_expd", [96, IMAGE_W], f32,
                                     kind="ExternalOutput")

    with tile.TileContext(nc) as tc:
        with ExitStack() as ctx:
            sb = ctx.enter_context(tc.tile_pool(name="sb", bufs=1))
            ps = ctx.enter_context(tc.tile_pool(name="ps", bufs=1, space="PSUM"))

            # ---- canvas slice (w-major) + small consts early on sync ----
            wsrc = w_in[:].rearrange("h (f w) k -> w f h k", f=2)
            wraw = sb.tile([WH, 2, HC, 3], f32, tag="wraw")
            for hf in range(2):
                nc.sync.dma_start(out=wraw[:, hf], in_=wsrc[:, hf])
            prep = sb.tile([128, 2, 3], f16, tag="prep")
            nc.sync.dma_start(out=prep[:], in_=prep_in[:])
            ident = sb.tile([128, 128], f32, tag="ident")
            nc.sync.dma_start(out=ident[:], in_=id_in[:])
            # big / late-needed consts on the scalar HWDGE ring
            b4 = sb.tile([WH, HC * 64], f32, tag="b4")
            nc.scalar.dma_start(out=b4[:], in_=b4_in[:])
            rt3 = sb.tile([96, ORC], f16, tag="rt3")
            nc.sync.dma_start(out=rt3[:], in_=rt3_in[:])
            esb = sb.tile([WH, 2, IMAGE_W], f16, tag="esb")
            for hf in range(2):
                nc.scalar.dma_start(out=esb[:, hf], in_=e_in[:, hf])
            ident16 = sb.tile([112, 112], f16, tag="ident16")
            nc.vector.tensor_copy(out=ident16[:], in_=ident[0:112, 0:112])

            waug = sb.tile([WH, 2, HC, 4], f32, tag="waug")
            for hf in range(2):
                nc.scalar.activation(
                    out=waug[:, hf, :, 0:3], in_=wraw[:, hf],
                    func=mybir.ActivationFunctionType.Sigmoid)
            nc.vector.memset(waug[:, :, :, 3:4], 1.0)

            # ---- W4g[4h+k, w] per half via fp32 PE transpose ------------
            w4g = []
            for hf in range(2):
                tp = ps.tile([112, WH], f32, tag="psA", bufs=2)
                nc.tensor.transpose(
                    out=tp[:], in_=waug[:, hf].rearrange("w h k -> w (h k)"),
                    identity=ident[0:WH, 0:112])
                g = sb.tile([112, WH], f32, tag=f"w4g{hf}")
                nc.vector.tensor_copy(out=g[:], in_=tp[:])
                w4g.append(g)

            # ---- v via block-diagonal fp32 matmuls, argmax straight ----
            # from PSUM: reduce_max + is_equal read the matmul result in
            # place, so the v matrix is never copied to SBUF
            vmax = sb.tile([WH, 2, HC], f32, tag="vmax")
            oh = sb.tile([WH, 2, HC, NUM_COLORS], f16, tag="oh")
            for hf in range(2):
                for g in range(4):
                    nh = min(8, HC - 8 * g)
                    nn = 64 * nh
                    sp = ps.tile([WH, 512], f32, tag="psS", bufs=2)
                    nc.tensor.matmul(
                        out=sp[:, 0:nn], lhsT=w4g[hf][:],
                        rhs=b4[:, 512 * g:512 * g + nn],
                        start=True, stop=True)
                    spv = sp[:, 0:nn].rearrange("w (h c) -> w h c", c=64)
                    vm = vmax[:, hf, 8 * g:8 * g + nh]
                    nc.vector.tensor_reduce(
                        out=vm, in_=spv, axis=mybir.AxisListType.X,
                        op=ALU.max)
                    nc.vector.tensor_tensor(
                        out=oh[:, hf, 8 * g:8 * g + nh], in0=spv,
                        in1=vm.unsqueeze(2).to_broadcast([WH, nh, 64]),
                        op=ALU.is_equal)

            # ---- transpose one-hot (fp16): oht[64*dh + c, hf, j, w] -----
            oht = sb.tile([128, 2, HC // 2, WH], f16, tag="oht")
            for hf in range(2):
                for jc in range(4):
                    j0, nj = 4 * jc, min(4, HC // 2 - 4 * jc)
                    tp = ps.tile([128, 4 * WH], f16, tag="psA", bufs=2)
                    for j in range(j0, j0 + nj):
                        nc.tensor.transpose(
                            out=tp[:, (j - j0) * WH:(j - j0 + 1) * WH],
                            in_=oh[:, hf, 2 * j:2 * j + 2]
                            .rearrange("w h c -> w (h c)"),
                            identity=ident16[:, 0:112])
                    eng = nc.vector if (jc % 2 == 0) else nc.scalar
                    dst = (oht[:, hf, j0:j0 + nj]
                           .rearrange("c j w -> c (j w)"))
                    if eng is nc.vector:
                        eng.tensor_copy(out=dst, in_=tp[:, 0:nj * WH])
                    else:
                        eng.copy(out=dst, in_=tp[:, 0:nj * WH])

            # ---- colors^T via palette-stationary fp16 split matmuls -----
            # ctsb[ch, hf, h, w]; h = 2j + dh
            ctsb = sb.tile([3, 2, HC, WH], f32, tag="ctsb")
            for hf in range(2):
                for dh in range(2):
                    for jc in range(4):
                        j0, nj = 4 * jc, min(4, HC // 2 - 4 * jc)
                        cp = ps.tile([3, 448], f32, tag="psB", bufs=4)
                        rhs = (oht[64 * dh:64 * dh + 64, hf, j0:j0 + nj]
                               .rearrange("c j w -> c (j w)"))
                        for part in range(2):
                            nc.tensor.matmul(
                                out=cp[:, 0:nj * WH],
                                lhsT=prep[64 * dh:64 * dh + 64, part, :],
                                rhs=rhs,
                                start=(part == 0), stop=(part == 1))
                        eng = nc.vector if (jc % 2 == 0) else nc.scalar
                        dst = ctsb[:, hf, 2 * j0 + dh:2 * (j0 + nj - 1) + dh + 1:2]
                        src3 = cp[:, 0:nj * WH].rearrange("c (j w) -> c j w", w=WH)
                        if eng is nc.vector:
                            eng.tensor_copy(out=dst, in_=src3)
                        else:
                            eng.copy(out=dst, in_=src3)

            # ---- colors to w-partitions, split to fp16 hi/lo in the ----
            # PSUM->SBUF copies: hi = f16(x) via cast copy, lo = f16(x - hi)
            # via a mixed-dtype subtract. pos = 4*b + a for h = 7*a + b.
            cw16 = sb.tile([WH, 2, 2, 3, 32], f16, tag="cw16")
            nc.vector.memset(cw16[:], 0.0)
            for hf in range(2):
                for b in range(7):
                    tp = ps.tile([WH, 4, 3], f32, tag="psA", bufs=2)
                    for a in range(4):
                        h = 7 * a + b
                        nc.tensor.transpose(
                            out=tp[:, a, :], in_=ctsb[:, hf, h, :],
                            identity=ident[0:3, 0:3])
                    srcv = tp[:].rearrange("w a c -> w c a")
                    hi = cw16[:, hf, 0, :, 4 * b:4 * b + 4]
                    eng = nc.vector if (b % 2 == 0) else nc.scalar
                    if eng is nc.vector:
                        eng.tensor_copy(out=hi, in_=srcv)
                    else:
                        eng.copy(out=hi, in_=srcv)
                    nc.vector.tensor_sub(
                        out=cw16[:, hf, 1, :, 4 * b:4 * b + 4],
                        in0=srcv, in1=hi)

            # ---- column expansion producing fp16 hi/lo directly --------
            # expd16 hi = sum_w cw16_hi * E (exact f16 values in f32 PSUM,
            # cast back exactly); interleaved per 512-column chunk with the
            # row-replication matmuls and per-half-channel stores.
            expd16 = sb.tile([96, 2, IMAGE_W], f16, tag="expd16")
            ofs = [sb.tile([128, 2, IMAGE_W], f32, tag=f"of{ch}", name=f"of{ch}")
                   for ch in range(3)]
            for jc in range(4):
                sl = slice(jc * 512, (jc + 1) * 512)
                for part in range(2):
                    ep = ps.tile([96, 512], f32, tag="psB", bufs=4)
                    for hf in range(2):
                        nc.tensor.matmul(
                            out=ep[:],
                            lhsT=cw16[:, hf, part].rearrange("w c p -> w (c p)"),
                            rhs=esb[:, hf, sl],
                            start=(hf == 0), stop=(hf == 1))
                    eng = nc.vector if (part == 0) else nc.scalar
                    if eng is nc.vector:
                        eng.tensor_copy(out=expd16[:, part, sl], in_=ep[:])
                    else:
                        eng.copy(out=expd16[:, part, sl], in_=ep[:])
                for ch in range(3):
                    for hf2 in range(2):
                        rp = ps.tile([128, 512], f32, tag="psB", bufs=4)
                        for part in range(2):
                            nc.tensor.matmul(
                                out=rp[:],
                                lhsT=rt3[32 * ch:32 * ch + 28,
                                         128 * hf2:128 * hf2 + 128],
                                rhs=expd16[32 * ch:32 * ch + 28, part, sl],
                                start=(part == 0), stop=(part == 1))
                        eng = nc.vector if ((ch + hf2) % 2 == 0) else nc.scalar
                        if eng is nc.vector:
                            eng.tensor_copy(out=ofs[ch][:, hf2, sl], in_=rp[:])
                        else:
                            eng.copy(out=ofs[ch][:, hf2, sl], in_=rp[:])
            if debug:
                dbg16 = sb.tile([96, IMAGE_W], f32, tag="dbg16")
                nc.vector.tensor_copy(out=dbg16[:], in_=expd16[:, 0])
                nc.scalar.add(out=dbg16[:], in_=expd16[:, 1], add=dbg16[:, 0:1])
                nc.sync.dma_start(out=dbg["expd"][:], in_=dbg16[:])
            for ch in range(3):
                for hf2 in range(2):
                    dma_eng = nc.sync if ((ch + hf2) % 2 == 0) else nc.scalar
                    dma_eng.dma_start(
                        out=out[ch, 128 * hf2:128 * hf2 + 128, :],
                        in_=ofs[ch][:, hf2])

    nc.compile()
    return nc, ["w_in", "b4_in", "prep_in", "rt3_in", "id_in", "e_in"]


def _host_consts(palette: np.ndarray):
    pal = palette.astype(np.float32)
    # block-diagonal distance matrix: rows (28h x 4k), cols (28h x 64c)
    b4row = np.empty((4, NUM_COLORS), np.float32)
    b4row[0:3] = -pal.T
    b4row[3] = 0.5 * (pal.astype(np.float64) ** 2).sum(-1).astype(np.float32)
    b4 = np.zeros((WH, HC * NUM_COLORS), np.float32)
    for h in range(HC):
        b4[4 * h:4 * h + 4, 64 * h:64 * h + 64] = b4row
    # palette hi/lo fp16 split, doubled along partitions
    hi = pal.astype(np.float16)
    lo = (pal - hi.astype(np.float32)).astype(np.float16)
    prep = np.stack([hi, lo], axis=1)                # (64, 2, 3)
    prep = np.concatenate([prep, prep], axis=0)      # (128, 2, 3)
    # row-replication matrix (0/1, fp16-exact), tripled for base partitions
    rowmap = (np.arange(ORC) * CANVAS_H) // IMAGE_H
    posmap = 4 * (rowmap % 7) + rowmap // 7
    rt = (posmap[None, :] == np.arange(32)[:, None]).astype(np.float16)
    rt3 = np.concatenate([rt, rt, rt], axis=0)       # (96, 256)
    # column-expansion matrix (0/1, fp16-exact), w split into two K-halves
    wmap = (np.arange(IMAGE_W) * CANVAS_W) // IMAGE_W
    e_full = (wmap[None, :] == np.arange(CANVAS_W)[:, None]).astype(np.float16)
    e = np.ascontiguousarray(
        np.stack([e_full[:WH], e_full[WH:]], axis=1))  # (112, 2, 2048)
    ident = np.eye(128, dtype=np.float32)
    return b4, prep, rt3, e, ident


def kernel(weight_logits, palette, image_h, image_w):
    weight_logits = np.asarray(weight_logits, np.float32)
    palette = np.asarray(palette, np.float32)
    assert int(image_h) == IMAGE_H and int(image_w) == IMAGE_W
    assert weight_logits.shape == (CANVAS_H, CANVAS_W, 3)

    if "nc" not in _CACHE:
        _CACHE["nc"] = _build_program()
    nc, _ = _CACHE["nc"]

    from concourse import bass_utils

    b4, prep, rt3, e, ident = _host_consts(palette)
    in_maps = []
    for core in range(N_CORES):
        sl = weight_logits[core * HC:(core + 1) * HC]
        in_maps.append({
            "w_in": np.ascontiguousarray(sl),
            "b4_in": b4, "prep_in": prep, "rt3_in": rt3,
            "id_in": ident, "e_in": e,
        })
    res = bass_utils.run_bass_kernel_spmd(
        nc, in_maps, core_ids=list(range(N_CORES)))
    outs = [res.results[c]["out"] for c in range(N_CORES)]
    return np.concatenate(outs, axis=1)
